# revision 46
# baseline (speedup 1.0000x reference)
"""Trainium2 Bass kernel for nn_AttentionHead_Hybrid2 (B=4, N=4096, DK=64).

reference:
    V = x @ Wv.T + bv              (B,N,DK)
    Q = x @ wq ; K = x @ wk        (B,N)
    A = exp(-(Q_i - K_j)^2)        (B,N,N)
    P = softmax(A / 8, axis=-1)
    out = LN(P @ V + x)

Sharding: 8 cores = (batch b = c//2) x (query half c%2). Each core gets the
full key/value set for its batch (rolled so its 2048 queries are rows 0:2048)
and produces its 2048x64 output slice.

Key idea: the score between query i and key j depends on j ONLY through the
scalar K_j. Keys are binned onto a uniform M-point grid over K-space with
linear (hat-function) interpolation, which is exact to O(delta^2) and whose
per-key errors oscillate in sign and wash out over 4096 keys:
    e(Q_i,K_j) ~= sum_m w_jm e(Q_i,kappa_m),  w_jm = hat((K_j-kappa_m)/delta)
so, with xa = [x | 1]:
    out_i = sum_j e_ij [V|1]_j = sum_m E(Q_i,kappa_m) * bva_m
    bva   = (W.T @ xa) @ [Wv.T|..; bv|..]         (bin-major "binned V")
collapsing the (2048 x 4096) score work to (2048 x M) plus cheap binning.

Precision strategy: every PE matmul runs in bf16 (4x the fp32 rate), with
hi/lo splitting wherever bf16 quantization would show: x is shipped as
bf16 hi + lo residual pairs (natural and transposed), E is shipped to the
PE as E-1 in bf16 (the +1 folds into the softmax ratio via the exact bin
column-sums), and binned-V is split hi/lo on chip. The hat weight is
w = 1 - min(|iota' + n_j|, 1) (one ACT Abs + one DVE min per key tile,
scale/offset/clamp baked into host constants); the "1 -" folds out of the
binning matmul through the G ones-column, cancelling exactly.

Phases: prep (K cols, Q row, q_rep outer) -> bin (G += xa.T @ minU) ->
score (E chunks: Square/Exp/Exp + accT += bva.T @ E') -> finish (transpose,
ratio, +x, LayerNorm), finish pipelined in two query halves.
"""

import sys

for _p in ("/opt/trn_rl_repo", "/root/.axon_site/_ro/trn_rl_repo"):
    if _p not in sys.path:
        sys.path.insert(0, _p)

import numpy as np

import concourse.bass as bass
import concourse.mybir as mybir
import concourse.tile as tile
import bass_rust
from concourse.bass_utils import run_bass_kernel_spmd

F32 = mybir.dt.float32
BF16 = mybir.dt.bfloat16
I32 = mybir.dt.int32
AF = mybir.ActivationFunctionType
OP = mybir.AluOpType

B, N, DK = 4, 4096, 64
NQ = 2048          # queries per core
NCORES = 8
JT = N // 128      # 32 key tiles
IT = NQ // 128     # 16 query tiles
M = 256            # K-grid bins
MC = M // 128      # bin chunks
K0 = -5.5
DELTA = 11.0 / (M - 1)
EPS = 1e-5

# packed const blob layout (128 partitions wide)
_IDENT0 = 0
_KAP0 = 128
_GAM0 = _KAP0 + MC
_BET0 = _GAM0 + DK
BLOB_W = _BET0 + DK


def split_multiwaits(nc):
    """Walrus in this env accepts one sem-wait per instruction; Tile emits
    several. Split extras onto preceding same-engine NoOps."""
    ctr = 0
    for f in nc.m.functions:
        for bb in f.blocks:
            out, changed = [], False
            for ins in bb.instructions:
                si = ins.sync_info
                if si is not None and si.on_wait and len(si.on_wait) > 1:
                    waits = list(si.on_wait)
                    for w in waits[:-1]:
                        ctr += 1
                        out.append(mybir.InstNoOp(
                            name=f"I-wsplit-{ctr}", engine=ins.engine,
                            debug=ins.debug, ins=[], outs=[],
                            sync_info=bass_rust.SyncInfo(on_wait=[w], on_update=[])))
                    ins.sync_info = bass_rust.SyncInfo(
                        on_wait=[waits[-1]], on_update=list(si.on_update or []))
                    changed = True
                out.append(ins)
            if changed:
                bb.instructions = out
    return ctr


def build_nc(split=True):
    nc = bass.Bass("TRN2", target_bir_lowering=False, debug=False)

    xa_d = nc.dram_tensor("xa", [N, 65], BF16, kind="ExternalInput").ap()
    xl_d = nc.dram_tensor("xl", [N, 65], BF16, kind="ExternalInput").ap()
    xth_d = nc.dram_tensor("xth", [DK, N], BF16, kind="ExternalInput").ap()
    xtl_d = nc.dram_tensor("xtl", [DK, N], BF16, kind="ExternalInput").ap()
    wvkb_d = nc.dram_tensor("wvkb", [DK + 1, 67], F32, kind="ExternalInput").ap()
    blob_d = nc.dram_tensor("blob", [128, BLOB_W], F32, kind="ExternalInput").ap()
    out_d = nc.dram_tensor("out", [NQ, DK], F32, kind="ExternalOutput").ap()

    with tile.TileContext(nc) as tc:
        cpool = tc.alloc_tile_pool(name="consts", bufs=1)
        big = tc.alloc_tile_pool(name="big", bufs=1)

        wvkb = cpool.tile([DK + 1, 67], F32)
        nc.sync.dma_start(wvkb[:], wvkb_d[:])
        wvkh = cpool.tile([DK + 1, 67], BF16)
        nc.vector.tensor_copy(wvkh[:], wvkb[:])
        wvkl = cpool.tile([DK + 1, 67], BF16)
        nc.vector.tensor_tensor(wvkl[:], wvkb[:], wvkh[:], OP.subtract)

        xth = big.tile([DK, N], BF16)
        xtl = big.tile([DK, N], BF16)

        blob = cpool.tile([128, BLOB_W], F32)
        ident = blob[:, _IDENT0:_IDENT0 + 128]
        kap = blob[:, _KAP0:_KAP0 + MC]
        gam = blob[:, _GAM0:_GAM0 + DK]
        bet = blob[:, _BET0:_BET0 + DK]

        eps_c = cpool.tile([128, 2], F32)
        nc.gpsimd.memset(eps_c[:], EPS)
        nc.scalar.activation(eps_c[:, 1:2], eps_c[:, 0:1], AF.Abs, scale=1.0)
        ones_f = cpool.tile([1, 128], F32)
        nc.gpsimd.memset(ones_f[:], 1.0)
        # iota' = m + K0/DELTA, built on device; cols M, M+1 are sentinels that
        # force hat weight 0 there, making the matmul's last columns pure
        # ones-columns (w = 1 - min(|big|,1) ... min saturates to 1? no: the
        # sentinel forces u>=1 so min=1; those columns carry sum(xa*1) after
        # the 1-complement, i.e. they become the exact xsum columns)
        ioti = cpool.tile([128, M + 2], I32)
        nc.gpsimd.iota(ioti[:], [[1, M + 2]], channel_multiplier=0)
        iota = cpool.tile([128, M + 2], F32)
        nc.vector.tensor_scalar(iota[:], ioti[:], 1.0, K0 / DELTA, OP.mult, OP.add)
        nc.gpsimd.memset(iota[:, M:M + 2], 1.0e6)

        xa_all = big.tile([128, JT * 65], BF16)      # [x | 1] natural, bf16 hi
        xa_v = xa_all.rearrange("p (t c) -> p t c", c=65)
        xl_all = big.tile([128, JT * 65], BF16)      # natural lo residual
        xl_v = xl_all.rearrange("p (t c) -> p t c", c=65)
        for h in range(4):
            nc.gpsimd.dma_start(
                xa_v[:, h * 8:(h + 1) * 8, :],
                xa_d[h * 1024:(h + 1) * 1024, :].rearrange("(t p) c -> p t c", p=128))
            nc.gpsimd.dma_start(
                xl_v[:, h * 8:(h + 1) * 8, :],
                xl_d[h * 1024:(h + 1) * 1024, :].rearrange("(t p) c -> p t c", p=128))
        # PE warmup: dense dummy matmuls while DMAs land, so the HAM clock
        # gate opens (1.2 -> 2.4 GHz) before the real matmuls begin
        with tc.tile_pool(name="warm_ps", bufs=1, space="PSUM") as wmp:
            wt = wmp.tile([128, 64], F32)
            for _ in range(24):
                nc.tensor.matmul(wt[:], iota[:, 0:128], iota[:, 0:64],
                                 start=True, stop=True)

        tcol = big.tile([128, JT], F32)              # hat bias -clamp(K/d ...)
        q_sb = big.tile([1, NQ], F32)
        q_rep = big.tile([128, NQ], F32)             # Q replicated across partitions
        ep_full = big.tile([128, MC * NQ], BF16)     # E-1 per bin chunk (bf16)
        ep_v = ep_full.rearrange("p (t i) -> p t i", i=NQ)

        with tc.tile_pool(name="prep_ps", bufs=2, space="PSUM") as pps:
            for h in range(8):
                c0, c1 = h * 512, (h + 1) * 512
                eng = nc.sync if h % 2 == 0 else nc.scalar
                eng.dma_start(xth[:, c0:c1], xth_d[:, c0:c1])
                eng2 = nc.scalar if h % 2 == 0 else nc.sync
                eng2.dma_start(xtl[:, c0:c1], xtl_d[:, c0:c1])
                for jt in range(h * 4, (h + 1) * 4):
                    kc = pps.tile([128, 1], F32, tag="kc")
                    nc.tensor.matmul(kc[:], xth[:, jt * 128:(jt + 1) * 128],
                                     wvkh[0:DK, 65:66], start=True, stop=True)
                    # bias n = -clamp(K/d, K0/d, K0/d + M-1); kc holds -K/d
                    nc.vector.tensor_scalar(tcol[:, jt:jt + 1], kc[:],
                                            -K0 / DELTA - (M - 1), -K0 / DELTA,
                                            OP.max, OP.min)
                if h >= 4:
                    continue
                # Q row chunk (hi+lo accumulated)
                qp = pps.tile([1, 512], F32, tag="qp")
                nc.tensor.matmul(qp[:], wvkh[0:DK, 66:67], xth[:, c0:c1],
                                 start=True, stop=False)
                nc.tensor.matmul(qp[:], wvkl[0:DK, 66:67], xtl[:, c0:c1],
                                 start=False, stop=True)
                nc.vector.tensor_copy(q_sb[0:1, c0:c1], qp[:])
            # blob (kap/ident/gamma/beta): queued behind the critical chunks
            nc.scalar.dma_start(blob[:], blob_d[:])
            # replicate Q across partitions via fp32 ones-outer
            for ic in range(NQ // 512):
                qr = pps.tile([128, 512], F32, tag="qr")
                nc.tensor.matmul(qr[:], ones_f[0:1, :],
                                 q_sb[0:1, ic * 512:(ic + 1) * 512],
                                 start=True, stop=True)
                nc.vector.tensor_copy(q_rep[:, ic * 512:(ic + 1) * 512], qr[:])

        def emit_e_chunk(mc, ep_):
            sq = ep_.tile([128, NQ], F32, tag="sq")
            nc.scalar.activation(sq[:], q_rep[:], AF.Square,
                                 bias=kap[:, mc:mc + 1], scale=-1.0)
            a_t = ep_.tile([128, NQ], F32, tag="a")
            nc.scalar.activation(a_t[:], sq[:], AF.Exp, scale=-1.0)
            e_t = ep_.tile([128, NQ], F32, tag="e")
            nc.scalar.activation(e_t[:], a_t[:], AF.Exp, scale=0.125)
            # E' = E - 1 in bf16 for the PE (the +1 rides the bin colsums)
            nc.vector.tensor_scalar(ep_v[:, mc, :], e_t[:], -1.0, None, OP.add)

        # ---- binning + interleaved score passes ----
        H = big.tile([DK + 1, M], F32)
        with tc.tile_pool(name="g_ps", bufs=1, space="PSUM") as gp:
            G = gp.tile([DK + 1, M + 2], F32)        # cols M,M+1 = min-ones
            with (tc.tile_pool(name="w_sb", bufs=4) as wp,
                  tc.tile_pool(name="e_scr", bufs=2) as ep_):
                for jt in range(JT):
                    u_t = wp.tile([128, M + 2], BF16, tag="u")
                    nc.scalar.activation(u_t[:], iota[:], AF.Abs,
                                         bias=tcol[:, jt:jt + 1], scale=1.0)
                    w_t = wp.tile([128, M + 2], BF16, tag="w")
                    nc.vector.tensor_scalar(w_t[:], u_t[:], 1.0, None, OP.min)
                    nc.tensor.matmul(G[:], xa_v[:, jt, :], w_t[:],
                                     start=(jt == 0), stop=False)
                    nc.tensor.matmul(G[:], xl_v[:, jt, :], w_t[:],
                                     start=False, stop=(jt == JT - 1))
                    if jt in (17, 25):
                        emit_e_chunk((jt - 17) // 8, ep_)
            # col M of G now holds sum_j min(big,1)*xa = xsum exactly
            G_sb = big.tile([DK + 1, M + 1], F32)
            nc.vector.tensor_copy(G_sb[:], G[:, 0:M + 1])
            # hat = 1 - min(u,1) => binned xa = xsum - G, far terms cancel
            nc.vector.tensor_tensor(H[:], G_sb[:, M:M + 1].broadcast_to([DK + 1, M]),
                                    G_sb[:, 0:M], OP.subtract)

        # bva = H.T @ wvkb  (bin-major binned [V|count]), split hi/lo bf16
        bvah = big.tile([128, MC * 65], BF16)
        bvah_v = bvah.rearrange("p (t c) -> p t c", c=65)
        bval = big.tile([128, MC * 65], BF16)
        bval_v = bval.rearrange("p (t c) -> p t c", c=65)
        colsum = big.tile([65, 1], F32)
        with tc.tile_pool(name="bva_ps", bufs=2, space="PSUM") as bp:
            for mc in range(MC):
                bt = bp.tile([128, 65], F32, tag="bt")
                nc.tensor.matmul(bt[:], H[:, mc * 128:(mc + 1) * 128],
                                 wvkb[:, 0:65], start=True, stop=True)
                nc.vector.tensor_copy(bvah_v[:, mc, :], bt[:])
                nc.vector.tensor_tensor(bval_v[:, mc, :], bt[:], bvah_v[:, mc, :],
                                        OP.subtract)
            # colsum of Vaug = G ones-column pushed through the weights
            cs = bp.tile([65, 1], F32, tag="cs")
            nc.tensor.matmul(cs[:], wvkb[:, 0:65], G_sb[0:DK + 1, M:M + 1],
                             start=True, stop=True)
            nc.vector.tensor_copy(colsum[:], cs[:])

        # ---- score matmuls + pipelined finish (two query halves) ----
        with tc.tile_pool(name="acc_ps", bufs=1, space="PSUM") as accp:
            accT = accp.tile([65, NQ], F32)          # 4 banks
            outT = big.tile([65, NQ], F32)
            nat = big.tile([128, IT * 65], F32)
            nat_v = nat.rearrange("p (t c) -> p t c", c=65)
            fin = big.tile([128, IT * DK], F32)
            fin_v = fin.rearrange("p (t d) -> p t d", d=DK)
            xq = big.tile([128, IT * DK], F32)
            xq_v = xq.rearrange("p (t d) -> p t d", d=DK)
            nc.vector.tensor_tensor(xq_v[:], xa_v[:, 0:IT, 0:DK],
                                    xl_v[:, 0:IT, 0:DK], OP.add)
            rec = big.tile([128, IT], F32)
            stat = big.tile([128, 4 * IT], F32)
            scr = big.tile([128, IT * DK], F32)
            scr_v = scr.rearrange("p (t d) -> p t d", d=DK)

            NH = 4                                   # finish pipeline chunks
            HT = IT // NH
            with tc.tile_pool(name="fin_ps", bufs=3, space="PSUM") as finp:
                for h in range(NH):
                    i0, i1 = h * (NQ // NH), (h + 1) * (NQ // NH)
                    for c in range(i0 // 512, i1 // 512):
                        for mc in range(MC):
                            sl = slice(c * 512, (c + 1) * 512)
                            nc.tensor.matmul(accT[:, sl], bvah_v[:, mc, :],
                                             ep_v[:, mc, sl],
                                             start=(mc == 0), stop=False)
                            nc.tensor.matmul(accT[:, sl], bval_v[:, mc, :],
                                             ep_v[:, mc, sl],
                                             start=False, stop=(mc == MC - 1))
                    # outT = accT + colsum  (restores the +1 of E = 1 + E')
                    nc.vector.tensor_tensor(
                        outT[:, i0:i1], accT[:, i0:i1],
                        colsum.broadcast_to([65, NQ // NH]), OP.add)
                    for it in range(h * HT, (h + 1) * HT):
                        np_t = finp.tile([128, 65], F32, tag="nat")
                        nc.tensor.transpose(np_t[:], outT[:, it * 128:(it + 1) * 128],
                                            ident[0:65, 0:65])
                        nc.vector.tensor_copy(nat_v[:, it, :], np_t[:])

                    ts_, te_ = h * HT, (h + 1) * HT
                    n_v = nat_v[:, ts_:te_, :]
                    f_v = fin_v[:, ts_:te_, :]
                    s_v = scr_v[:, ts_:te_, :]
                    sum_ = stat[:, 0 * IT + ts_:0 * IT + te_]
                    m_ = stat[:, 1 * IT + ts_:1 * IT + te_]
                    v_ = stat[:, 2 * IT + ts_:2 * IT + te_]
                    rstd = stat[:, 3 * IT + ts_:3 * IT + te_]
                    rc = rec[:, ts_:te_]

                    nc.vector.reciprocal(rc, n_v[:, :, 64])
                    nc.vector.tensor_tensor(
                        f_v, n_v[:, :, 0:DK],
                        rc.unsqueeze(-1).broadcast_to([128, HT, DK]), OP.mult)
                    nc.vector.tensor_tensor(f_v, f_v, xq_v[:, ts_:te_, :], OP.add)
                    nc.vector.reduce_sum(sum_, f_v, axis=mybir.AxisListType.X)
                    nc.vector.tensor_scalar_mul(m_, sum_, 1.0 / DK)
                    nc.vector.tensor_tensor(
                        f_v, f_v, m_.unsqueeze(-1).broadcast_to([128, HT, DK]),
                        OP.subtract)
                    nc.scalar.activation(s_v.rearrange("p t d -> p (t d)"),
                                         f_v.rearrange("p t d -> p (t d)"),
                                         AF.Square, scale=1.0)
                    nc.vector.reduce_sum(v_, s_v, axis=mybir.AxisListType.X)
                    nc.scalar.activation(rstd, v_, AF.Ln, bias=eps_c[:, 0:1],
                                         scale=1.0 / DK)
                    nc.scalar.activation(rstd, rstd, AF.Exp, scale=-0.5)
                    nc.vector.tensor_tensor(
                        f_v, f_v, rstd.unsqueeze(-1).broadcast_to([128, HT, DK]),
                        OP.mult)
                    nc.vector.tensor_tensor(
                        f_v, f_v, gam.unsqueeze(1).broadcast_to([128, HT, DK]),
                        OP.mult)
                    nc.vector.tensor_tensor(
                        f_v, f_v, bet.unsqueeze(1).broadcast_to([128, HT, DK]),
                        OP.add)
                    nc.sync.dma_start(
                        out_d[i0:i1, :].rearrange("(t p) d -> p t d", p=128), f_v)

        big.release()
        cpool.release()

    if split:
        split_multiwaits(nc)
    return nc


_NC_CACHE = None


def _get_nc():
    global _NC_CACHE
    if _NC_CACHE is None:
        _NC_CACHE = build_nc()
    return _NC_CACHE


def make_in_maps(x, Wv, bv, wq, wk, gamma, beta):
    import ml_dtypes
    x = np.asarray(x, np.float32)
    wkp = (np.asarray(wk, np.float64) * (-1.0 / DELTA)).astype(np.float32)
    wvk = np.concatenate([np.asarray(Wv, np.float32).T,
                          np.zeros((DK, 1), np.float32),
                          wkp[:, None],
                          np.asarray(wq, np.float32)[:, None]], axis=1)
    brow = np.concatenate([np.asarray(bv, np.float32), [1.0, 0.0, 0.0]]
                          ).astype(np.float32)
    wvkb = np.concatenate([wvk, brow[None, :]], axis=0).copy()      # (65, 67)

    blob = np.zeros((128, BLOB_W), np.float32)
    blob[:, _IDENT0:_IDENT0 + 128] = np.eye(128, dtype=np.float32)
    kgrid = (K0 + DELTA * np.arange(M, dtype=np.float64)).astype(np.float32)
    blob[:, _KAP0:_KAP0 + MC] = kgrid.reshape(MC, 128).T
    blob[:, _GAM0:_GAM0 + DK] = np.asarray(gamma, np.float32)[None, :]
    blob[:, _BET0:_BET0 + DK] = np.asarray(beta, np.float32)[None, :]

    ones = np.ones((N, 1), np.float32)
    in_maps = []
    for c in range(NCORES):
        b, qoff = c // 2, (c % 2) * NQ
        xr = np.concatenate([x[b, qoff:], x[b, :qoff]], axis=0) if qoff else x[b]
        xaf = np.concatenate([xr, ones], 1)
        xa = xaf.astype(ml_dtypes.bfloat16)
        xl = (xaf - xa.astype(np.float32)).astype(ml_dtypes.bfloat16)
        xtf = np.ascontiguousarray(xr.T)
        xth = xtf.astype(ml_dtypes.bfloat16)
        xtl = (xtf - xth.astype(np.float32)).astype(ml_dtypes.bfloat16)
        in_maps.append({"xa": np.ascontiguousarray(xa),
                        "xl": np.ascontiguousarray(xl),
                        "xth": np.ascontiguousarray(xth),
                        "xtl": np.ascontiguousarray(xtl),
                        "wvkb": wvkb, "blob": blob})
    return in_maps


def kernel(x, Wv, bv, wq, wk, gamma, beta, _trace=False, _trace_cores=None):
    nc = _get_nc()
    in_maps = make_in_maps(x, Wv, bv, wq, wk, gamma, beta)
    res = run_bass_kernel_spmd(nc, in_maps, core_ids=list(range(NCORES)),
                               trace=_trace, trace_cores=_trace_cores)
    out = np.empty((B, N, DK), np.float32)
    for c in range(NCORES):
        b, qoff = c // 2, (c % 2) * NQ
        out[b, qoff:qoff + NQ] = res.results[c]["out"]
    kernel._last_results = res
    return out


# revision 47
# speedup vs baseline: 1.0808x; 1.0808x over previous
"""Trainium2 Bass kernel for nn_AttentionHead_Hybrid2 (B=4, N=4096, DK=64).

reference:
    V = x @ Wv.T + bv              (B,N,DK)
    Q = x @ wq ; K = x @ wk        (B,N)
    A = exp(-(Q_i - K_j)^2)        (B,N,N)
    P = softmax(A / 8, axis=-1)
    out = LN(P @ V + x)

Sharding: 8 cores = (batch b = c//2) x (query half c%2). Each core gets the
full key/value set for its batch (rolled so its 2048 queries are rows 0:2048)
and produces its 2048x64 output slice.

Key idea: the score between query i and key j depends on j ONLY through the
scalar K_j. Keys are binned onto a uniform M-point grid over K-space with
linear (hat-function) interpolation, which is exact to O(delta^2) and whose
per-key errors oscillate in sign and wash out over 4096 keys:
    e(Q_i,K_j) ~= sum_m w_jm e(Q_i,kappa_m),  w_jm = hat((K_j-kappa_m)/delta)
so, with xa = [x | 1]:
    out_i = sum_j e_ij [V|1]_j = sum_m E(Q_i,kappa_m) * bva_m
    bva   = (W.T @ xa) @ [Wv.T|..; bv|..]         (bin-major "binned V")
collapsing the (2048 x 4096) score work to (2048 x M) plus cheap binning.

Precision strategy: every PE matmul runs in bf16 (4x the fp32 rate), with
hi/lo splitting wherever bf16 quantization would show: x is shipped as
bf16 hi + lo residual pairs (natural and transposed), E is shipped to the
PE as E-1 in bf16 (the +1 folds into the softmax ratio via the exact bin
column-sums), and binned-V is split hi/lo on chip. The hat weight is
w = 1 - min(|iota' + n_j|, 1) (one ACT Abs + one DVE min per key tile,
scale/offset/clamp baked into host constants); the "1 -" folds out of the
binning matmul through the G ones-column, cancelling exactly.

Phases: prep (K cols, Q row, q_rep outer) -> bin (G += xa.T @ minU) ->
score (E chunks: Square/Exp/Exp + accT += bva.T @ E') -> finish (transpose,
ratio, +x, LayerNorm), finish pipelined in two query halves.
"""

import sys

for _p in ("/opt/trn_rl_repo", "/root/.axon_site/_ro/trn_rl_repo"):
    if _p not in sys.path:
        sys.path.insert(0, _p)

import numpy as np

import concourse.bass as bass
import concourse.mybir as mybir
import concourse.tile as tile
import bass_rust
from concourse.bass_utils import run_bass_kernel_spmd

F32 = mybir.dt.float32
BF16 = mybir.dt.bfloat16
I32 = mybir.dt.int32
AF = mybir.ActivationFunctionType
OP = mybir.AluOpType

B, N, DK = 4, 4096, 64
NQ = 2048          # queries per core
NCORES = 8
JT = N // 128      # 32 key tiles
IT = NQ // 128     # 16 query tiles
M = 256            # K-grid bins
MC = M // 128      # bin chunks
K0 = -5.5
DELTA = 11.0 / (M - 1)
EPS = 1e-5

# packed const blob layout (128 partitions wide)
_IDENT0 = 0
_KAP0 = 128
_GAM0 = _KAP0 + MC
_BET0 = _GAM0 + DK
BLOB_W = _BET0 + DK


def split_multiwaits(nc):
    """Walrus in this env accepts one sem-wait per instruction; Tile emits
    several. Split extras onto preceding same-engine NoOps."""
    ctr = 0
    for f in nc.m.functions:
        for bb in f.blocks:
            out, changed = [], False
            for ins in bb.instructions:
                si = ins.sync_info
                if si is not None and si.on_wait and len(si.on_wait) > 1:
                    waits = list(si.on_wait)
                    for w in waits[:-1]:
                        ctr += 1
                        out.append(mybir.InstNoOp(
                            name=f"I-wsplit-{ctr}", engine=ins.engine,
                            debug=ins.debug, ins=[], outs=[],
                            sync_info=bass_rust.SyncInfo(on_wait=[w], on_update=[])))
                    ins.sync_info = bass_rust.SyncInfo(
                        on_wait=[waits[-1]], on_update=list(si.on_update or []))
                    changed = True
                out.append(ins)
            if changed:
                bb.instructions = out
    return ctr


def build_nc(split=True):
    nc = bass.Bass("TRN2", target_bir_lowering=False, debug=False)

    xa_d = nc.dram_tensor("xa", [N, 65], BF16, kind="ExternalInput").ap()
    xl_d = nc.dram_tensor("xl", [N, 65], BF16, kind="ExternalInput").ap()
    xth_d = nc.dram_tensor("xth", [DK, N], BF16, kind="ExternalInput").ap()
    xtl_d = nc.dram_tensor("xtl", [DK, N], BF16, kind="ExternalInput").ap()
    wvkb_d = nc.dram_tensor("wvkb", [DK + 1, 67], F32, kind="ExternalInput").ap()
    blob_d = nc.dram_tensor("blob", [128, BLOB_W], F32, kind="ExternalInput").ap()
    out_d = nc.dram_tensor("out", [NQ, DK], F32, kind="ExternalOutput").ap()

    with tile.TileContext(nc) as tc:
        cpool = tc.alloc_tile_pool(name="consts", bufs=1)
        big = tc.alloc_tile_pool(name="big", bufs=1)

        wvkb = cpool.tile([DK + 1, 67], F32)
        nc.sync.dma_start(wvkb[:], wvkb_d[:])
        wvkh = cpool.tile([DK + 1, 67], BF16)
        nc.vector.tensor_copy(wvkh[:], wvkb[:])
        wvkl = cpool.tile([DK + 1, 67], BF16)
        nc.vector.tensor_tensor(wvkl[:], wvkb[:], wvkh[:], OP.subtract)

        xth = big.tile([DK, N], BF16)
        xtl = big.tile([DK, N], BF16)

        blob = cpool.tile([128, BLOB_W], F32)
        ident = blob[:, _IDENT0:_IDENT0 + 128]
        kap = blob[:, _KAP0:_KAP0 + MC]
        gam = blob[:, _GAM0:_GAM0 + DK]
        bet = blob[:, _BET0:_BET0 + DK]

        eps_c = cpool.tile([128, 2], F32)
        nc.gpsimd.memset(eps_c[:], EPS)
        nc.scalar.activation(eps_c[:, 1:2], eps_c[:, 0:1], AF.Abs, scale=1.0)
        ones_f = cpool.tile([1, 128], F32)
        nc.gpsimd.memset(ones_f[:], 1.0)
        # iota' = m + K0/DELTA, built on device; cols M, M+1 are sentinels that
        # force hat weight 0 there, making the matmul's last columns pure
        # ones-columns (w = 1 - min(|big|,1) ... min saturates to 1? no: the
        # sentinel forces u>=1 so min=1; those columns carry sum(xa*1) after
        # the 1-complement, i.e. they become the exact xsum columns)
        ioti = cpool.tile([128, M + 2], I32)
        nc.gpsimd.iota(ioti[:], [[1, M + 2]], channel_multiplier=0)
        iota = cpool.tile([128, M + 2], F32)
        nc.vector.tensor_scalar(iota[:], ioti[:], 1.0, K0 / DELTA, OP.mult, OP.add)
        nc.gpsimd.memset(iota[:, M:M + 2], 1.0e6)

        xa_all = big.tile([128, JT * 65], BF16)      # [x | 1] natural, bf16 hi
        xa_v = xa_all.rearrange("p (t c) -> p t c", c=65)
        xl_all = big.tile([128, JT * 65], BF16)      # natural lo residual
        xl_v = xl_all.rearrange("p (t c) -> p t c", c=65)
        for h in range(4):
            nc.gpsimd.dma_start(
                xa_v[:, h * 8:(h + 1) * 8, :],
                xa_d[h * 1024:(h + 1) * 1024, :].rearrange("(t p) c -> p t c", p=128))
            nc.gpsimd.dma_start(
                xl_v[:, h * 8:(h + 1) * 8, :],
                xl_d[h * 1024:(h + 1) * 1024, :].rearrange("(t p) c -> p t c", p=128))
        # PE warmup: dense dummy matmuls while the x DMAs land, so the HAM
        # clock gate opens (1.2 -> 2.4 GHz) just before the real matmuls;
        # wvkh arrives ~when the first x chunk does, timing the burst right
        with tc.tile_pool(name="warm_ps", bufs=1, space="PSUM") as wmp:
            wt = wmp.tile([65, 64], F32)
            for _ in range(36):
                nc.tensor.matmul(wt[:], wvkh[:, 0:65], wvkh[:, 0:64],
                                 start=True, stop=True)

        tcol = big.tile([128, JT], F32)              # hat bias -clamp(K/d ...)
        q_sb = big.tile([1, NQ], F32)
        q_rep = big.tile([128, NQ], F32)             # Q replicated across partitions
        ep_full = big.tile([128, MC * NQ], BF16)     # E-1 per bin chunk (bf16)
        ep_v = ep_full.rearrange("p (t i) -> p t i", i=NQ)

        with tc.tile_pool(name="prep_ps", bufs=2, space="PSUM") as pps:
            for h in range(8):
                c0, c1 = h * 512, (h + 1) * 512
                eng = nc.sync if h % 2 == 0 else nc.scalar
                eng.dma_start(xth[:, c0:c1], xth_d[:, c0:c1])
                eng2 = nc.scalar if h % 2 == 0 else nc.sync
                eng2.dma_start(xtl[:, c0:c1], xtl_d[:, c0:c1])
                for jt in range(h * 4, (h + 1) * 4):
                    kc = pps.tile([128, 1], F32, tag="kc")
                    nc.tensor.matmul(kc[:], xth[:, jt * 128:(jt + 1) * 128],
                                     wvkh[0:DK, 65:66], start=True, stop=True)
                    # bias n = -clamp(K/d, K0/d, K0/d + M-1); kc holds -K/d
                    nc.vector.tensor_scalar(tcol[:, jt:jt + 1], kc[:],
                                            -K0 / DELTA - (M - 1), -K0 / DELTA,
                                            OP.max, OP.min)
                if h >= 4:
                    continue
                # Q row chunk (hi+lo accumulated)
                qp = pps.tile([1, 512], F32, tag="qp")
                nc.tensor.matmul(qp[:], wvkh[0:DK, 66:67], xth[:, c0:c1],
                                 start=True, stop=False)
                nc.tensor.matmul(qp[:], wvkl[0:DK, 66:67], xtl[:, c0:c1],
                                 start=False, stop=True)
                nc.vector.tensor_copy(q_sb[0:1, c0:c1], qp[:])
            # blob (kap/ident/gamma/beta): queued behind the critical chunks
            nc.scalar.dma_start(blob[:], blob_d[:])
            # replicate Q across partitions via fp32 ones-outer
            for ic in range(NQ // 512):
                qr = pps.tile([128, 512], F32, tag="qr")
                nc.tensor.matmul(qr[:], ones_f[0:1, :],
                                 q_sb[0:1, ic * 512:(ic + 1) * 512],
                                 start=True, stop=True)
                nc.vector.tensor_copy(q_rep[:, ic * 512:(ic + 1) * 512], qr[:])

        def emit_e_chunk(mc, ep_):
            sq = ep_.tile([128, NQ], F32, tag="sq")
            nc.scalar.activation(sq[:], q_rep[:], AF.Square,
                                 bias=kap[:, mc:mc + 1], scale=-1.0)
            a_t = ep_.tile([128, NQ], F32, tag="a")
            nc.scalar.activation(a_t[:], sq[:], AF.Exp, scale=-1.0)
            e_t = ep_.tile([128, NQ], F32, tag="e")
            nc.scalar.activation(e_t[:], a_t[:], AF.Exp, scale=0.125)
            # E' = E - 1 in bf16 for the PE (the +1 rides the bin colsums)
            nc.vector.tensor_scalar(ep_v[:, mc, :], e_t[:], -1.0, None, OP.add)

        # ---- binning + interleaved score passes ----
        H = big.tile([DK + 1, M], F32)
        with tc.tile_pool(name="g_ps", bufs=1, space="PSUM") as gp:
            G = gp.tile([DK + 1, M + 2], F32)        # cols M,M+1 = min-ones
            with (tc.tile_pool(name="w_sb", bufs=4) as wp,
                  tc.tile_pool(name="e_scr", bufs=2) as ep_):
                for jt in range(JT):
                    u_t = wp.tile([128, M + 2], BF16, tag="u")
                    nc.scalar.activation(u_t[:], iota[:], AF.Abs,
                                         bias=tcol[:, jt:jt + 1], scale=1.0)
                    w_t = wp.tile([128, M + 2], BF16, tag="w")
                    nc.vector.tensor_scalar(w_t[:], u_t[:], 1.0, None, OP.min)
                    nc.tensor.matmul(G[:], xa_v[:, jt, :], w_t[:],
                                     start=(jt == 0), stop=False)
                    nc.tensor.matmul(G[:], xl_v[:, jt, :], w_t[:],
                                     start=False, stop=(jt == JT - 1))
                    if jt in (17, 25):
                        emit_e_chunk((jt - 17) // 8, ep_)
            # col M of G now holds sum_j min(big,1)*xa = xsum exactly
            G_sb = big.tile([DK + 1, M + 1], F32)
            nc.vector.tensor_copy(G_sb[:], G[:, 0:M + 1])
            # hat = 1 - min(u,1) => binned xa = xsum - G, far terms cancel
            nc.vector.tensor_tensor(H[:], G_sb[:, M:M + 1].broadcast_to([DK + 1, M]),
                                    G_sb[:, 0:M], OP.subtract)

        # bva = H.T @ wvkb  (bin-major binned [V|count]), split hi/lo bf16
        bvah = big.tile([128, MC * 65], BF16)
        bvah_v = bvah.rearrange("p (t c) -> p t c", c=65)
        bval = big.tile([128, MC * 65], BF16)
        bval_v = bval.rearrange("p (t c) -> p t c", c=65)
        colsum = big.tile([65, 1], F32)
        with tc.tile_pool(name="bva_ps", bufs=2, space="PSUM") as bp:
            for mc in range(MC):
                bt = bp.tile([128, 65], F32, tag="bt")
                nc.tensor.matmul(bt[:], H[:, mc * 128:(mc + 1) * 128],
                                 wvkb[:, 0:65], start=True, stop=True)
                nc.vector.tensor_copy(bvah_v[:, mc, :], bt[:])
                nc.vector.tensor_tensor(bval_v[:, mc, :], bt[:], bvah_v[:, mc, :],
                                        OP.subtract)
            # colsum of Vaug = G ones-column pushed through the weights
            cs = bp.tile([65, 1], F32, tag="cs")
            nc.tensor.matmul(cs[:], wvkb[:, 0:65], G_sb[0:DK + 1, M:M + 1],
                             start=True, stop=True)
            nc.vector.tensor_copy(colsum[:], cs[:])

        # ---- score matmuls + pipelined finish (two query halves) ----
        with tc.tile_pool(name="acc_ps", bufs=4, space="PSUM") as accp:
            outT = big.tile([65, NQ], F32)
            nat = big.tile([128, IT * 65], F32)
            nat_v = nat.rearrange("p (t c) -> p t c", c=65)
            fin = big.tile([128, IT * DK], F32)
            fin_v = fin.rearrange("p (t d) -> p t d", d=DK)
            xq = big.tile([128, IT * DK], F32)
            xq_v = xq.rearrange("p (t d) -> p t d", d=DK)
            nc.vector.tensor_tensor(xq_v[:], xa_v[:, 0:IT, 0:DK],
                                    xl_v[:, 0:IT, 0:DK], OP.add)
            rec = big.tile([128, IT], F32)
            stat = big.tile([128, 4 * IT], F32)
            scr = big.tile([128, IT * DK], F32)
            scr_v = scr.rearrange("p (t d) -> p t d", d=DK)

            NH = 4                                   # finish pipeline chunks
            HT = IT // NH
            with tc.tile_pool(name="fin_ps", bufs=3, space="PSUM") as finp:
                for h in range(NH):
                    i0, i1 = h * (NQ // NH), (h + 1) * (NQ // NH)
                    accT = accp.tile([65, NQ // NH], F32, tag="acc")
                    for mc in range(MC):
                        nc.tensor.matmul(accT[:], bvah_v[:, mc, :],
                                         ep_v[:, mc, i0:i1],
                                         start=(mc == 0), stop=False)
                        nc.tensor.matmul(accT[:], bval_v[:, mc, :],
                                         ep_v[:, mc, i0:i1],
                                         start=False, stop=(mc == MC - 1))
                    # outT = accT + colsum  (restores the +1 of E = 1 + E')
                    nc.vector.tensor_tensor(
                        outT[:, i0:i1], accT[:],
                        colsum.broadcast_to([65, NQ // NH]), OP.add)
                    for it in range(h * HT, (h + 1) * HT):
                        np_t = finp.tile([128, 65], F32, tag="nat")
                        nc.tensor.transpose(np_t[:], outT[:, it * 128:(it + 1) * 128],
                                            ident[0:65, 0:65])
                        nc.vector.tensor_copy(nat_v[:, it, :], np_t[:])

                    ts_, te_ = h * HT, (h + 1) * HT
                    n_v = nat_v[:, ts_:te_, :]
                    f_v = fin_v[:, ts_:te_, :]
                    s_v = scr_v[:, ts_:te_, :]
                    sum_ = stat[:, 0 * IT + ts_:0 * IT + te_]
                    m_ = stat[:, 1 * IT + ts_:1 * IT + te_]
                    v_ = stat[:, 2 * IT + ts_:2 * IT + te_]
                    rstd = stat[:, 3 * IT + ts_:3 * IT + te_]
                    rc = rec[:, ts_:te_]

                    nc.vector.reciprocal(rc, n_v[:, :, 64])
                    nc.vector.tensor_tensor(
                        f_v, n_v[:, :, 0:DK],
                        rc.unsqueeze(-1).broadcast_to([128, HT, DK]), OP.mult)
                    nc.vector.tensor_tensor(f_v, f_v, xq_v[:, ts_:te_, :], OP.add)
                    nc.vector.reduce_sum(sum_, f_v, axis=mybir.AxisListType.X)
                    nc.vector.tensor_scalar_mul(m_, sum_, 1.0 / DK)
                    nc.vector.tensor_tensor(
                        f_v, f_v, m_.unsqueeze(-1).broadcast_to([128, HT, DK]),
                        OP.subtract)
                    nc.scalar.activation(s_v.rearrange("p t d -> p (t d)"),
                                         f_v.rearrange("p t d -> p (t d)"),
                                         AF.Square, scale=1.0)
                    nc.vector.reduce_sum(v_, s_v, axis=mybir.AxisListType.X)
                    nc.scalar.activation(rstd, v_, AF.Ln, bias=eps_c[:, 0:1],
                                         scale=1.0 / DK)
                    nc.scalar.activation(rstd, rstd, AF.Exp, scale=-0.5)
                    nc.vector.tensor_tensor(
                        f_v, f_v, rstd.unsqueeze(-1).broadcast_to([128, HT, DK]),
                        OP.mult)
                    nc.vector.tensor_tensor(
                        f_v, f_v, gam.unsqueeze(1).broadcast_to([128, HT, DK]),
                        OP.mult)
                    nc.vector.tensor_tensor(
                        f_v, f_v, bet.unsqueeze(1).broadcast_to([128, HT, DK]),
                        OP.add)
                    nc.sync.dma_start(
                        out_d[i0:i1, :].rearrange("(t p) d -> p t d", p=128), f_v)

        big.release()
        cpool.release()

    if split:
        split_multiwaits(nc)
    return nc


_NC_CACHE = None


def _get_nc():
    global _NC_CACHE
    if _NC_CACHE is None:
        _NC_CACHE = build_nc()
    return _NC_CACHE


def make_in_maps(x, Wv, bv, wq, wk, gamma, beta):
    import ml_dtypes
    x = np.asarray(x, np.float32)
    wkp = (np.asarray(wk, np.float64) * (-1.0 / DELTA)).astype(np.float32)
    wvk = np.concatenate([np.asarray(Wv, np.float32).T,
                          np.zeros((DK, 1), np.float32),
                          wkp[:, None],
                          np.asarray(wq, np.float32)[:, None]], axis=1)
    brow = np.concatenate([np.asarray(bv, np.float32), [1.0, 0.0, 0.0]]
                          ).astype(np.float32)
    wvkb = np.concatenate([wvk, brow[None, :]], axis=0).copy()      # (65, 67)

    blob = np.zeros((128, BLOB_W), np.float32)
    blob[:, _IDENT0:_IDENT0 + 128] = np.eye(128, dtype=np.float32)
    kgrid = (K0 + DELTA * np.arange(M, dtype=np.float64)).astype(np.float32)
    blob[:, _KAP0:_KAP0 + MC] = kgrid.reshape(MC, 128).T
    blob[:, _GAM0:_GAM0 + DK] = np.asarray(gamma, np.float32)[None, :]
    blob[:, _BET0:_BET0 + DK] = np.asarray(beta, np.float32)[None, :]

    ones = np.ones((N, 1), np.float32)
    in_maps = []
    for c in range(NCORES):
        b, qoff = c // 2, (c % 2) * NQ
        xr = np.concatenate([x[b, qoff:], x[b, :qoff]], axis=0) if qoff else x[b]
        xaf = np.concatenate([xr, ones], 1)
        xa = xaf.astype(ml_dtypes.bfloat16)
        xl = (xaf - xa.astype(np.float32)).astype(ml_dtypes.bfloat16)
        xtf = np.ascontiguousarray(xr.T)
        xth = xtf.astype(ml_dtypes.bfloat16)
        xtl = (xtf - xth.astype(np.float32)).astype(ml_dtypes.bfloat16)
        in_maps.append({"xa": np.ascontiguousarray(xa),
                        "xl": np.ascontiguousarray(xl),
                        "xth": np.ascontiguousarray(xth),
                        "xtl": np.ascontiguousarray(xtl),
                        "wvkb": wvkb, "blob": blob})
    return in_maps


def kernel(x, Wv, bv, wq, wk, gamma, beta, _trace=False, _trace_cores=None):
    nc = _get_nc()
    in_maps = make_in_maps(x, Wv, bv, wq, wk, gamma, beta)
    res = run_bass_kernel_spmd(nc, in_maps, core_ids=list(range(NCORES)),
                               trace=_trace, trace_cores=_trace_cores)
    out = np.empty((B, N, DK), np.float32)
    for c in range(NCORES):
        b, qoff = c // 2, (c % 2) * NQ
        out[b, qoff:qoff + NQ] = res.results[c]["out"]
    kernel._last_results = res
    return out


# revision 48
# speedup vs baseline: 1.1057x; 1.0230x over previous
"""Trainium2 Bass kernel for nn_AttentionHead_Hybrid2 (B=4, N=4096, DK=64).

reference:
    V = x @ Wv.T + bv              (B,N,DK)
    Q = x @ wq ; K = x @ wk        (B,N)
    A = exp(-(Q_i - K_j)^2)        (B,N,N)
    P = softmax(A / 8, axis=-1)
    out = LN(P @ V + x)

Sharding: 8 cores = (batch b = c//2) x (query half c%2). Each core gets the
full key/value set for its batch (rolled so its 2048 queries are rows 0:2048)
and produces its 2048x64 output slice.

Key idea: the score between query i and key j depends on j ONLY through the
scalar K_j. Keys are binned onto a uniform M-point grid over K-space with
linear (hat-function) interpolation, which is exact to O(delta^2) and whose
per-key errors oscillate in sign and wash out over 4096 keys:
    e(Q_i,K_j) ~= sum_m w_jm e(Q_i,kappa_m),  w_jm = hat((K_j-kappa_m)/delta)
so, with xa = [x | 1]:
    out_i = sum_j e_ij [V|1]_j = sum_m E(Q_i,kappa_m) * bva_m
    bva   = (W.T @ xa) @ [Wv.T|..; bv|..]         (bin-major "binned V")
collapsing the (2048 x 4096) score work to (2048 x M) plus cheap binning.

Precision strategy: every PE matmul runs in bf16 (4x the fp32 rate), with
hi/lo splitting wherever bf16 quantization would show: x is shipped as
bf16 hi + lo residual pairs (natural and transposed), E is shipped to the
PE as E-1 in bf16 (the +1 folds into the softmax ratio via the exact bin
column-sums), and binned-V is split hi/lo on chip. The hat weight is
w = 1 - min(|iota' + n_j|, 1) (one ACT Abs + one DVE min per key tile,
scale/offset/clamp baked into host constants); the "1 -" folds out of the
binning matmul through the G ones-column, cancelling exactly.

Phases: prep (K cols, Q row, q_rep outer) -> bin (G += xa.T @ minU) ->
score (E chunks: Square/Exp/Exp + accT += bva.T @ E') -> finish (transpose,
ratio, +x, LayerNorm), finish pipelined in two query halves.
"""

import sys

for _p in ("/opt/trn_rl_repo", "/root/.axon_site/_ro/trn_rl_repo"):
    if _p not in sys.path:
        sys.path.insert(0, _p)

import numpy as np

import concourse.bass as bass
import concourse.mybir as mybir
import concourse.tile as tile
import bass_rust
from concourse.bass_utils import run_bass_kernel_spmd

F32 = mybir.dt.float32
BF16 = mybir.dt.bfloat16
I32 = mybir.dt.int32
AF = mybir.ActivationFunctionType
OP = mybir.AluOpType

B, N, DK = 4, 4096, 64
NQ = 2048          # queries per core
NCORES = 8
JT = N // 128      # 32 key tiles
IT = NQ // 128     # 16 query tiles
M = 256            # K-grid bins
MC = M // 128      # bin chunks
K0 = -5.5
DELTA = 11.0 / (M - 1)
EPS = 1e-5

# packed const blob layout (128 partitions wide)
_IDENT0 = 0
_KAP0 = 128
_GAM0 = _KAP0 + MC
_BET0 = _GAM0 + DK
BLOB_W = _BET0 + DK


def split_multiwaits(nc):
    """Walrus in this env accepts one sem-wait per instruction; Tile emits
    several. Split extras onto preceding same-engine NoOps."""
    ctr = 0
    for f in nc.m.functions:
        for bb in f.blocks:
            out, changed = [], False
            for ins in bb.instructions:
                si = ins.sync_info
                if si is not None and si.on_wait and len(si.on_wait) > 1:
                    waits = list(si.on_wait)
                    for w in waits[:-1]:
                        ctr += 1
                        out.append(mybir.InstNoOp(
                            name=f"I-wsplit-{ctr}", engine=ins.engine,
                            debug=ins.debug, ins=[], outs=[],
                            sync_info=bass_rust.SyncInfo(on_wait=[w], on_update=[])))
                    ins.sync_info = bass_rust.SyncInfo(
                        on_wait=[waits[-1]], on_update=list(si.on_update or []))
                    changed = True
                out.append(ins)
            if changed:
                bb.instructions = out
    return ctr


def build_nc(split=True):
    nc = bass.Bass("TRN2", target_bir_lowering=False, debug=False)

    xa_d = nc.dram_tensor("xa", [N, 65], BF16, kind="ExternalInput").ap()
    xl_d = nc.dram_tensor("xl", [NQ, 65], BF16, kind="ExternalInput").ap()
    xth_d = nc.dram_tensor("xth", [DK, N], BF16, kind="ExternalInput").ap()
    xtl_d = nc.dram_tensor("xtl", [DK, N], BF16, kind="ExternalInput").ap()
    wvkb_d = nc.dram_tensor("wvkb", [DK + 1, 67], F32, kind="ExternalInput").ap()
    blob_d = nc.dram_tensor("blob", [128, BLOB_W], F32, kind="ExternalInput").ap()
    out_d = nc.dram_tensor("out", [NQ, DK], F32, kind="ExternalOutput").ap()

    with tile.TileContext(nc) as tc:
        cpool = tc.alloc_tile_pool(name="consts", bufs=1)
        big = tc.alloc_tile_pool(name="big", bufs=1)

        wvkb = cpool.tile([DK + 1, 67], F32)
        nc.sync.dma_start(wvkb[:], wvkb_d[:])
        wvkh = cpool.tile([DK + 1, 67], BF16)
        nc.vector.tensor_copy(wvkh[:], wvkb[:])
        wvkl = cpool.tile([DK + 1, 67], BF16)
        nc.vector.tensor_tensor(wvkl[:], wvkb[:], wvkh[:], OP.subtract)

        xth = big.tile([DK, N], BF16)
        xtl = big.tile([DK, N], BF16)

        blob = cpool.tile([128, BLOB_W], F32)
        ident = blob[:, _IDENT0:_IDENT0 + 128]
        kap = blob[:, _KAP0:_KAP0 + MC]
        gam = blob[:, _GAM0:_GAM0 + DK]
        bet = blob[:, _BET0:_BET0 + DK]

        eps_c = cpool.tile([128, 2], F32)
        nc.gpsimd.memset(eps_c[:], EPS)
        nc.scalar.activation(eps_c[:, 1:2], eps_c[:, 0:1], AF.Abs, scale=1.0)
        ones_f = cpool.tile([1, 128], F32)
        nc.gpsimd.memset(ones_f[:], 1.0)
        # iota' = m + K0/DELTA, built on device; cols M, M+1 are sentinels that
        # force hat weight 0 there, making the matmul's last columns pure
        # ones-columns (w = 1 - min(|big|,1) ... min saturates to 1? no: the
        # sentinel forces u>=1 so min=1; those columns carry sum(xa*1) after
        # the 1-complement, i.e. they become the exact xsum columns)
        ioti = cpool.tile([128, M + 2], I32)
        nc.gpsimd.iota(ioti[:], [[1, M + 2]], channel_multiplier=0)
        iota = cpool.tile([128, M + 2], F32)
        nc.vector.tensor_scalar(iota[:], ioti[:], 1.0, K0 / DELTA, OP.mult, OP.add)
        nc.gpsimd.memset(iota[:, M:M + 2], 1.0e6)

        xa_all = big.tile([128, JT * 65], BF16)      # [x | 1] natural, bf16 hi
        xa_v = xa_all.rearrange("p (t c) -> p t c", c=65)
        xl_all = big.tile([128, IT * 65], BF16)      # lo residual (queries)
        xl_v = xl_all.rearrange("p (t c) -> p t c", c=65)
        for h in range(4):
            nc.gpsimd.dma_start(
                xa_v[:, h * 8:(h + 1) * 8, :],
                xa_d[h * 1024:(h + 1) * 1024, :].rearrange("(t p) c -> p t c", p=128))
            if h < 2:
                nc.gpsimd.dma_start(
                    xl_v[:, h * 8:(h + 1) * 8, :],
                    xl_d[h * 1024:(h + 1) * 1024, :].rearrange("(t p) c -> p t c",
                                                               p=128))


        tcol = big.tile([128, JT], F32)              # hat bias -clamp(K/d ...)
        q_sb = big.tile([1, NQ], F32)
        q_rep = big.tile([128, NQ], F32)             # Q replicated across partitions
        ep_full = big.tile([128, MC * NQ], BF16)     # E-1 per bin chunk (bf16)
        ep_v = ep_full.rearrange("p (t i) -> p t i", i=NQ)

        with tc.tile_pool(name="prep_ps", bufs=2, space="PSUM") as pps:
            for h in range(8):
                c0, c1 = h * 512, (h + 1) * 512
                eng = nc.sync if h % 2 == 0 else nc.scalar
                eng.dma_start(xth[:, c0:c1], xth_d[:, c0:c1])
                eng2 = nc.scalar if h % 2 == 0 else nc.sync
                eng2.dma_start(xtl[:, c0:c1], xtl_d[:, c0:c1])
                for jt in range(h * 4, (h + 1) * 4):
                    kc = pps.tile([128, 1], F32, tag="kc")
                    nc.tensor.matmul(kc[:], xth[:, jt * 128:(jt + 1) * 128],
                                     wvkh[0:DK, 65:66], start=True, stop=True)
                    # bias n = -clamp(K/d, K0/d, K0/d + M-1); kc holds -K/d
                    nc.vector.tensor_scalar(tcol[:, jt:jt + 1], kc[:],
                                            -K0 / DELTA - (M - 1), -K0 / DELTA,
                                            OP.max, OP.min)
                if h >= 4:
                    continue
                # Q row chunk (hi+lo accumulated)
                qp = pps.tile([1, 512], F32, tag="qp")
                nc.tensor.matmul(qp[:], wvkh[0:DK, 66:67], xth[:, c0:c1],
                                 start=True, stop=False)
                nc.tensor.matmul(qp[:], wvkl[0:DK, 66:67], xtl[:, c0:c1],
                                 start=False, stop=True)
                nc.vector.tensor_copy(q_sb[0:1, c0:c1], qp[:])
            # blob (kap/ident/gamma/beta): queued behind the critical chunks
            nc.scalar.dma_start(blob[:], blob_d[:])
            # replicate Q across partitions via fp32 ones-outer
            for ic in range(NQ // 512):
                qr = pps.tile([128, 512], F32, tag="qr")
                nc.tensor.matmul(qr[:], ones_f[0:1, :],
                                 q_sb[0:1, ic * 512:(ic + 1) * 512],
                                 start=True, stop=True)
                nc.vector.tensor_copy(q_rep[:, ic * 512:(ic + 1) * 512], qr[:])

        def emit_e_chunk(mc, ep_):
            sq = ep_.tile([128, NQ], F32, tag="sq")
            nc.scalar.activation(sq[:], q_rep[:], AF.Square,
                                 bias=kap[:, mc:mc + 1], scale=-1.0)
            a_t = ep_.tile([128, NQ], F32, tag="a")
            nc.scalar.activation(a_t[:], sq[:], AF.Exp, scale=-1.0)
            e_t = ep_.tile([128, NQ], F32, tag="e")
            nc.scalar.activation(e_t[:], a_t[:], AF.Exp, scale=0.125)
            # E' = E - 1 in bf16 for the PE (the +1 rides the bin colsums)
            if mc % 2 == 0:
                nc.vector.tensor_scalar(ep_v[:, mc, :], e_t[:], -1.0, None, OP.add)
            else:
                nc.scalar.activation(ep_v[:, mc, :], e_t[:], AF.Copy, bias=-1.0)

        # ---- binning + interleaved score passes ----
        H = big.tile([DK + 1, M], F32)
        with tc.tile_pool(name="g_ps", bufs=1, space="PSUM") as gp:
            G = gp.tile([DK + 1, M + 2], F32)        # cols M,M+1 = min-ones
            with (tc.tile_pool(name="w_sb", bufs=4) as wp,
                  tc.tile_pool(name="e_scr", bufs=2) as ep_):
                for jt in range(JT):
                    u_t = wp.tile([128, M + 2], BF16, tag="u")
                    nc.scalar.activation(u_t[:], iota[:], AF.Abs,
                                         bias=tcol[:, jt:jt + 1], scale=1.0)
                    w_t = wp.tile([128, M + 2], BF16, tag="w")
                    nc.vector.tensor_scalar(w_t[:], u_t[:], 1.0, None, OP.min)
                    nc.tensor.matmul(G[:], xa_v[:, jt, :], w_t[:],
                                     start=(jt == 0), stop=(jt == JT - 1))
                    if jt in (17, 25):
                        emit_e_chunk((jt - 17) // 8, ep_)
            # col M of G now holds sum_j min(big,1)*xa = xsum exactly
            G_sb = big.tile([DK + 1, M + 1], F32)
            nc.vector.tensor_copy(G_sb[:], G[:, 0:M + 1])
            # hat = 1 - min(u,1) => binned xa = xsum - G, far terms cancel
            nc.vector.tensor_tensor(H[:], G_sb[:, M:M + 1].broadcast_to([DK + 1, M]),
                                    G_sb[:, 0:M], OP.subtract)

        # bva = H.T @ wvkb  (bin-major binned [V|count]), split hi/lo bf16
        bvah = big.tile([128, MC * 65], BF16)
        bvah_v = bvah.rearrange("p (t c) -> p t c", c=65)
        bval = big.tile([128, MC * 65], BF16)
        bval_v = bval.rearrange("p (t c) -> p t c", c=65)
        colsum = big.tile([65, 1], F32)
        with tc.tile_pool(name="bva_ps", bufs=2, space="PSUM") as bp:
            for mc in range(MC):
                bt = bp.tile([128, 65], F32, tag="bt")
                nc.tensor.matmul(bt[:], H[:, mc * 128:(mc + 1) * 128],
                                 wvkb[:, 0:65], start=True, stop=True)
                nc.vector.tensor_copy(bvah_v[:, mc, :], bt[:])
                nc.vector.tensor_tensor(bval_v[:, mc, :], bt[:], bvah_v[:, mc, :],
                                        OP.subtract)
            # colsum of Vaug = G ones-column pushed through the weights
            cs = bp.tile([65, 1], F32, tag="cs")
            nc.tensor.matmul(cs[:], wvkb[:, 0:65], G_sb[0:DK + 1, M:M + 1],
                             start=True, stop=True)
            nc.vector.tensor_copy(colsum[:], cs[:])

        # ---- score matmuls + pipelined finish (two query halves) ----
        with tc.tile_pool(name="acc_ps", bufs=4, space="PSUM") as accp:
            outT = big.tile([65, NQ], F32)
            nat = big.tile([128, IT * 65], F32)
            nat_v = nat.rearrange("p (t c) -> p t c", c=65)
            fin = big.tile([128, IT * DK], F32)
            fin_v = fin.rearrange("p (t d) -> p t d", d=DK)
            xq = big.tile([128, IT * DK], F32)
            xq_v = xq.rearrange("p (t d) -> p t d", d=DK)
            nc.vector.tensor_tensor(xq_v[:], xa_v[:, 0:IT, 0:DK],
                                    xl_v[:, 0:IT, 0:DK], OP.add)
            rec = big.tile([128, IT], F32)
            stat = big.tile([128, 4 * IT], F32)
            scr = big.tile([128, IT * DK], F32)
            scr_v = scr.rearrange("p (t d) -> p t d", d=DK)

            NH = 4                                   # finish pipeline chunks
            HT = IT // NH
            with tc.tile_pool(name="fin_ps", bufs=3, space="PSUM") as finp:
                for h in range(NH):
                    i0, i1 = h * (NQ // NH), (h + 1) * (NQ // NH)
                    accT = accp.tile([65, NQ // NH], F32, tag="acc")
                    for mc in range(MC):
                        nc.tensor.matmul(accT[:], bvah_v[:, mc, :],
                                         ep_v[:, mc, i0:i1],
                                         start=(mc == 0), stop=False)
                        nc.tensor.matmul(accT[:], bval_v[:, mc, :],
                                         ep_v[:, mc, i0:i1],
                                         start=False, stop=(mc == MC - 1))
                    # outT = accT + colsum  (restores the +1 of E = 1 + E')
                    nc.vector.tensor_tensor(
                        outT[:, i0:i1], accT[:],
                        colsum.broadcast_to([65, NQ // NH]), OP.add)
                    np_t = finp.tile([128, HT * 65], F32, tag="nat")
                    for q2 in range(HT):
                        it = h * HT + q2
                        nc.tensor.transpose(np_t[:, q2 * 65:(q2 + 1) * 65],
                                            outT[:, it * 128:(it + 1) * 128],
                                            ident[0:65, 0:65])
                    nc.vector.tensor_copy(nat_v[:, h * HT:(h + 1) * HT, :], np_t[:])

                    ts_, te_ = h * HT, (h + 1) * HT
                    n_v = nat_v[:, ts_:te_, :]
                    f_v = fin_v[:, ts_:te_, :]
                    s_v = scr_v[:, ts_:te_, :]
                    sum_ = stat[:, 0 * IT + ts_:0 * IT + te_]
                    m_ = stat[:, 1 * IT + ts_:1 * IT + te_]
                    v_ = stat[:, 2 * IT + ts_:2 * IT + te_]
                    rstd = stat[:, 3 * IT + ts_:3 * IT + te_]
                    rc = rec[:, ts_:te_]

                    nc.vector.reciprocal(rc, n_v[:, :, 64])
                    nc.vector.tensor_tensor(
                        f_v, n_v[:, :, 0:DK],
                        rc.unsqueeze(-1).broadcast_to([128, HT, DK]), OP.mult)
                    nc.vector.tensor_tensor(f_v, f_v, xq_v[:, ts_:te_, :], OP.add)
                    nc.vector.reduce_sum(sum_, f_v, axis=mybir.AxisListType.X)
                    nc.vector.tensor_scalar_mul(m_, sum_, 1.0 / DK)
                    nc.vector.tensor_tensor(
                        f_v, f_v, m_.unsqueeze(-1).broadcast_to([128, HT, DK]),
                        OP.subtract)
                    nc.scalar.activation(s_v.rearrange("p t d -> p (t d)"),
                                         f_v.rearrange("p t d -> p (t d)"),
                                         AF.Square, scale=1.0)
                    nc.vector.reduce_sum(v_, s_v, axis=mybir.AxisListType.X)
                    nc.scalar.activation(rstd, v_, AF.Ln, bias=eps_c[:, 0:1],
                                         scale=1.0 / DK)
                    nc.scalar.activation(rstd, rstd, AF.Exp, scale=-0.5)
                    nc.vector.tensor_tensor(
                        f_v, f_v, rstd.unsqueeze(-1).broadcast_to([128, HT, DK]),
                        OP.mult)
                    nc.vector.tensor_tensor(
                        f_v, f_v, gam.unsqueeze(1).broadcast_to([128, HT, DK]),
                        OP.mult)
                    nc.vector.tensor_tensor(
                        f_v, f_v, bet.unsqueeze(1).broadcast_to([128, HT, DK]),
                        OP.add)
                    nc.sync.dma_start(
                        out_d[i0:i1, :].rearrange("(t p) d -> p t d", p=128), f_v)

        big.release()
        cpool.release()

    if split:
        split_multiwaits(nc)
    return nc


_NC_CACHE = None


def _get_nc():
    global _NC_CACHE
    if _NC_CACHE is None:
        _NC_CACHE = build_nc()
    return _NC_CACHE


def make_in_maps(x, Wv, bv, wq, wk, gamma, beta):
    import ml_dtypes
    x = np.asarray(x, np.float32)
    wkp = (np.asarray(wk, np.float64) * (-1.0 / DELTA)).astype(np.float32)
    wvk = np.concatenate([np.asarray(Wv, np.float32).T,
                          np.zeros((DK, 1), np.float32),
                          wkp[:, None],
                          np.asarray(wq, np.float32)[:, None]], axis=1)
    brow = np.concatenate([np.asarray(bv, np.float32), [1.0, 0.0, 0.0]]
                          ).astype(np.float32)
    wvkb = np.concatenate([wvk, brow[None, :]], axis=0).copy()      # (65, 67)

    blob = np.zeros((128, BLOB_W), np.float32)
    blob[:, _IDENT0:_IDENT0 + 128] = np.eye(128, dtype=np.float32)
    kgrid = (K0 + DELTA * np.arange(M, dtype=np.float64)).astype(np.float32)
    blob[:, _KAP0:_KAP0 + MC] = kgrid.reshape(MC, 128).T
    blob[:, _GAM0:_GAM0 + DK] = np.asarray(gamma, np.float32)[None, :]
    blob[:, _BET0:_BET0 + DK] = np.asarray(beta, np.float32)[None, :]

    ones = np.ones((N, 1), np.float32)
    in_maps = []
    for c in range(NCORES):
        b, qoff = c // 2, (c % 2) * NQ
        xr = np.concatenate([x[b, qoff:], x[b, :qoff]], axis=0) if qoff else x[b]
        xaf = np.concatenate([xr, ones], 1)
        xa = xaf.astype(ml_dtypes.bfloat16)
        xl = (xaf[0:NQ] - xa[0:NQ].astype(np.float32)).astype(ml_dtypes.bfloat16)
        xtf = np.ascontiguousarray(xr.T)
        xth = xtf.astype(ml_dtypes.bfloat16)
        xtl = (xtf - xth.astype(np.float32)).astype(ml_dtypes.bfloat16)
        in_maps.append({"xa": np.ascontiguousarray(xa),
                        "xl": np.ascontiguousarray(xl),
                        "xth": np.ascontiguousarray(xth),
                        "xtl": np.ascontiguousarray(xtl),
                        "wvkb": wvkb, "blob": blob})
    return in_maps


def kernel(x, Wv, bv, wq, wk, gamma, beta, _trace=False, _trace_cores=None):
    nc = _get_nc()
    in_maps = make_in_maps(x, Wv, bv, wq, wk, gamma, beta)
    res = run_bass_kernel_spmd(nc, in_maps, core_ids=list(range(NCORES)),
                               trace=_trace, trace_cores=_trace_cores)
    out = np.empty((B, N, DK), np.float32)
    for c in range(NCORES):
        b, qoff = c // 2, (c % 2) * NQ
        out[b, qoff:qoff + NQ] = res.results[c]["out"]
    kernel._last_results = res
    return out
